# revision 47
# baseline (speedup 1.0000x reference)
"""Integrate-and-fire scan (T=8) on Trainium2, data-parallel over 8 NeuronCores.

Reference semantics per element, scanned over t:
    mem = mem + x[t]; spike = (mem - 1 > 0); mem = mem - spike

Sharding: batch dim (axis 1 of x / axis 0 of mem0) split 4-per-core across 8
cores; the scan is elementwise so no cross-core communication is needed.

Per core the shard is viewed as [T=8, P=128, F=4704] (4*3*224*224 = 602112 =
128*4704), split into 4 column chunks of 1176 that stay resident in SBUF as
membrane tiles. mem0 is folded into x[0] on the host (bit-exact), so device
traffic is just x in (19.3 MB) + spikes out (19.3 MB) per core — the HBM-stack
roofline (2 cores x 38.5 MB / 716 GB/s ~= 107 us).

Per timestep, phase-grouped across chunks so each in-order engine stream never
stalls: VectorE does mem += x[t] and mem -= spike (TT, 1x); ScalarE computes
spike = relu(sign(mem - 1)) (exactly (mem-1 > 0) in {0,1}); x streams in on
the sync HWDGE queue while spikes stream out via the gpsimd SWDGE queue.
Measured ~106-120 us on silicon, bit-exact vs the fp32 reference.
"""

import sys

if "/opt/trn_rl_repo" not in sys.path:
    sys.path.insert(0, "/opt/trn_rl_repo")

import numpy as np

import concourse.bass as bass  # noqa: F401  (registers engine classes)
import concourse.tile as tile
from concourse import bacc, mybir
from concourse.bass_utils import run_bass_kernel_spmd

T, B, C, H, W = 8, 32, 3, 224, 224
NCORES = 8
BPC = B // NCORES            # 4 batch elements per core
E = BPC * C * H * W          # 602112 elements per (core, timestep)
P = 128
F = E // P                   # 4704 free-dim columns
F32 = mybir.dt.float32
BF16 = mybir.dt.bfloat16

import os

# Tunables (env-overridable for A/B testing)
CHUNK_W = int(os.environ.get("IAF_CHUNK_W", "1176"))  # columns/chunk (divides 4704)
N_CHUNKS = F // CHUNK_W
X_BUFS = int(os.environ.get("IAF_X_BUFS", "12"))
S_BUFS = int(os.environ.get("IAF_S_BUFS", "8"))
SPIKE_ENGINE = os.environ.get("IAF_SPIKE", "scalar")  # "vector" | "scalar"
OUT_DMA_ENGINE = os.environ.get("IAF_OUT_DMA", "gpsimd")   # scalar|sync|gpsimd
MEM0_DMA_ENGINE = os.environ.get("IAF_MEM0_DMA", "scalar")  # scalar|sync|gpsimd
# On-device output dtype. Spikes are exactly {0,1}, which bf16/uint8 represent
# exactly, so the DRAM output can be narrow and the host upcasts to f32 —
# halving (bf16) or quartering (uint8) HBM write traffic with zero error.
U8 = mybir.dt.uint8
OUT_DT = {"f32": F32, "bf16": BF16, "u8": U8}[os.environ.get("IAF_OUT_DT", "u8")]
# Spike tile dtype in SBUF. "out" = same as OUT_DT (plain copy out-DMA);
# "f32" = keep f32 tiles (fast DVE sub operand) and let the SWDGE out-DMA
# cast down to OUT_DT.
SPIKE_TILE = os.environ.get("IAF_SPIKE_TILE", "out")
# Engine balance: every k-th spike (over the flat (t, c) index) computes as
# is_gt on DVE; the rest as sign+relu on ACT. 0 = all on ACT.
DVE_SPIKE_EVERY = int(os.environ.get("IAF_DVE_SPIKE_EVERY", "8"))
# Split the t=0 loads/ops into half-chunks to shorten the pipeline ramp.
T0_SPLIT = os.environ.get("IAF_T0_SPLIT", "1") == "1"
# Negated-state formulation: membrane tiles carry m-tilde = -mem between
# steps; u = x - m-tilde (TT), m-tilde' = spike - u via scalar_tensor_tensor
# reading the f32 sign scratch — DVE never touches the u8 spike tiles.
# Exact: spike - u and x - m-tilde round identically to the reference ops.
NEG_STATE = os.environ.get("IAF_NEG", "0") == "1"

_compiled_nc = None


def _build():
    nc = bacc.Bacc("TRN2", target_bir_lowering=False, debug=False,
                   num_devices=NCORES)
    # mem0 is folded into x[0] host-side (bit-exact: same fp32 add the device
    # would do), so the device reads only x and writes only spikes.
    x = nc.dram_tensor("x", [T, P, F], F32, kind="ExternalInput").ap()
    out = nc.dram_tensor("out", [T, P, F], OUT_DT, kind="ExternalOutput").ap()

    with tile.TileContext(nc) as tc:
        with tc.tile_pool(name="mem", bufs=N_CHUNKS) as mem_pool, \
             tc.tile_pool(name="xin", bufs=X_BUFS) as x_pool, \
             tc.tile_pool(name="sgn", bufs=4) as sg_pool, \
             tc.tile_pool(name="spk", bufs=S_BUFS) as s_pool:
            eng = {"scalar": nc.scalar, "sync": nc.sync, "gpsimd": nc.gpsimd}
            mem0_dma = eng[MEM0_DMA_ENGINE]
            if SPIKE_ENGINE == "scalar":
                # bias tile for Sign(u - 1); built on DVE inside the tile
                # graph so no manual barrier is needed.
                neg1 = mem_pool.tile([P, 1], F32)
                nc.vector.memset(neg1[:], -1.0)
            # t=0 membrane state is x'[0] = x[0] + mem0 (folded on host):
            # DMA it straight into the resident membrane tiles. With T0_SPLIT
            # the loads go in half-chunks alternating between the two HWDGE
            # queues so the first spike ops can start a few us earlier.
            mts = []
            for c in range(N_CHUNKS):
                mt = mem_pool.tile([P, CHUNK_W], F32)
                if T0_SPLIT:
                    hw = CHUNK_W // 2
                    for h in range(2):
                        dma = mem0_dma if (2 * c + h) % 2 == 0 else nc.sync
                        dma.dma_start(
                            out=mt[:, bass.ts(h, hw)],
                            in_=x[0, :, c * CHUNK_W + h * hw:
                                  c * CHUNK_W + (h + 1) * hw])
                else:
                    mem0_dma.dma_start(out=mt[:],
                                       in_=x[0, :, bass.ts(c, CHUNK_W)])
                mts.append(mt)
            out_dma = eng.get(OUT_DMA_ENGINE)
            # t-outer, chunk-inner, phase-grouped issue order: engines run
            # their instruction streams in order, so grouping each phase
            # across chunks keeps every engine stall-free (chunk c's spike
            # computes while chunk c+1 adds, etc).
            for t in range(T):
                sts = []
                if t > 0:
                    xts = []
                    for c in range(N_CHUNKS):
                        xt = x_pool.tile([P, CHUNK_W], F32)
                        nc.sync.dma_start(out=xt[:],
                                          in_=x[t, :, bass.ts(c, CHUNK_W)])
                        xts.append(xt)
                    for c in range(N_CHUNKS):
                        if NEG_STATE:
                            # u = x - m-tilde  (== x + mem)
                            nc.vector.tensor_sub(mts[c][:], xts[c][:], mts[c][:])
                        else:
                            nc.vector.tensor_add(mts[c][:], mts[c][:], xts[c][:])
                st_dt = F32 if SPIKE_TILE == "f32" else OUT_DT
                # Work items: (membrane AP, output col base, width). At t=0
                # with T0_SPLIT they are half-chunks so the pipeline ramps up
                # as soon as the first half-load lands.
                if t == 0 and T0_SPLIT:
                    hw = CHUNK_W // 2
                    items = [(mts[c][:, bass.ts(h, hw)], c * CHUNK_W + h * hw,
                              hw) for c in range(N_CHUNKS) for h in range(2)]
                else:
                    items = [(mts[c][:], c * CHUNK_W, CHUNK_W)
                             for c in range(N_CHUNKS)]
                sgs = []
                for i, (mem_ap, col, w) in enumerate(items):
                    st = s_pool.tile([P, w], st_dt)
                    sts.append(st)
                    # Offload every k-th spike to DVE, aligned to the END of
                    # each per-t spike phase (last chunk) so the swap never
                    # stalls ACT's stream mid-phase.
                    idx = t * N_CHUNKS + i
                    on_act = SPIKE_ENGINE == "scalar" and not (
                        DVE_SPIKE_EVERY
                        and (idx + 1) % DVE_SPIKE_EVERY == 0)
                    if on_act:
                        # spike = relu(sign(mem' - 1)) in {0,1}, exactly
                        # (u > 1); both steps on the otherwise-idle ScalarE.
                        # Sign goes to an f32 scratch (its {-1,0,1} range
                        # would wrap in uint8); Relu writes the spike dtype.
                        sg = sg_pool.tile([P, w], F32)
                        nc.scalar.activation(
                            sg[:], mem_ap, mybir.ActivationFunctionType.Sign,
                            bias=neg1[:], scale=1.0)
                        nc.scalar.activation(
                            st[:], sg[:], mybir.ActivationFunctionType.Relu)
                        sgs.append(sg)
                    else:
                        nc.vector.tensor_scalar(
                            out=st[:], in0=mem_ap, scalar1=1.0, scalar2=None,
                            op0=mybir.AluOpType.is_gt)
                        sgs.append(None)
                for i, (mem_ap, col, w) in enumerate(items):
                    if NEG_STATE:
                        if sgs[i] is not None:
                            # m-tilde = max(sg, 0) - u = spike - u
                            nc.vector.scalar_tensor_tensor(
                                out=mem_ap, in0=sgs[i][:], scalar=0.0,
                                in1=mem_ap, op0=mybir.AluOpType.max,
                                op1=mybir.AluOpType.subtract)
                        else:
                            # m-tilde = (u > 1) - u
                            nc.vector.scalar_tensor_tensor(
                                out=mem_ap, in0=mem_ap, scalar=1.0,
                                in1=mem_ap, op0=mybir.AluOpType.is_gt,
                                op1=mybir.AluOpType.subtract)
                    else:
                        nc.vector.tensor_sub(mem_ap, mem_ap, sts[i][:])
                    if OUT_DMA_ENGINE == "split":
                        dma_eng = nc.sync if i % 2 == 0 else nc.scalar
                    elif OUT_DMA_ENGINE == "mix":
                        dma_eng = nc.gpsimd if i % 2 == 0 else nc.sync
                    else:
                        dma_eng = out_dma
                    dma_eng.dma_start(out=out[t, :, col:col + w],
                                      in_=sts[i][:])
    nc.compile()
    return nc


def _get_nc():
    global _compiled_nc
    if _compiled_nc is None:
        _compiled_nc = _build()
    return _compiled_nc


def _run(x, mem0, trace=False):
    nc = _get_nc()
    in_maps = []
    for i in range(NCORES):
        bsl = slice(i * BPC, (i + 1) * BPC)
        xi = np.ascontiguousarray(x[:, bsl]).reshape(T, P, F)
        # Fold the initial membrane into the first timestep (bit-exact fp32
        # add, same rounding the device add would produce).
        xi[0] += mem0[bsl].reshape(P, F)
        in_maps.append({"x": xi})
    res = run_bass_kernel_spmd(nc, in_maps, list(range(NCORES)), trace=trace)
    full = np.empty((T, B, C, H, W), dtype=np.float32)
    for i in range(NCORES):
        shard = res.results[i]["out"].reshape(T, BPC, C, H, W)
        full[:, i * BPC:(i + 1) * BPC] = shard  # upcasts narrow spike dtypes
    return full, res


def kernel(x, mem0):
    x = np.asarray(x, dtype=np.float32)
    mem0 = np.asarray(mem0, dtype=np.float32)
    full, _ = _run(x, mem0, trace=False)
    return full


# revision 48
# speedup vs baseline: 1.0445x; 1.0445x over previous
"""Integrate-and-fire scan (T=8) on Trainium2, data-parallel over 8 NeuronCores.

Reference semantics per element, scanned over t:
    mem = mem + x[t]; spike = (mem - 1 > 0); mem = mem - spike

Sharding: batch dim (axis 1 of x / axis 0 of mem0) split 4-per-core across 8
cores; the scan is elementwise so no cross-core communication is needed.

Per core the shard is viewed as [T=8, P=128, F=4704] (4*3*224*224 = 602112 =
128*4704), split into 4 column chunks of 1176 that stay resident in SBUF as
membrane tiles. mem0 is folded into x[0] on the host (bit-exact), so device
traffic is just x in (19.3 MB) + spikes out (19.3 MB) per core — the HBM-stack
roofline (2 cores x 38.5 MB / 716 GB/s ~= 107 us).

Per timestep, phase-grouped across chunks so each in-order engine stream never
stalls: VectorE does mem += x[t] and mem -= spike (TT, 1x); ScalarE computes
spike = relu(sign(mem - 1)) (exactly (mem-1 > 0) in {0,1}); x streams in on
the sync HWDGE queue while spikes stream out via the gpsimd SWDGE queue.
Measured ~106-120 us on silicon, bit-exact vs the fp32 reference.
"""

import sys

if "/opt/trn_rl_repo" not in sys.path:
    sys.path.insert(0, "/opt/trn_rl_repo")

import numpy as np

import concourse.bass as bass  # noqa: F401  (registers engine classes)
import concourse.tile as tile
from concourse import bacc, mybir
from concourse.bass_utils import run_bass_kernel_spmd

T, B, C, H, W = 8, 32, 3, 224, 224
NCORES = 8
BPC = B // NCORES            # 4 batch elements per core
E = BPC * C * H * W          # 602112 elements per (core, timestep)
P = 128
F = E // P                   # 4704 free-dim columns
F32 = mybir.dt.float32
BF16 = mybir.dt.bfloat16

import os

# Tunables (env-overridable for A/B testing)
CHUNK_W = int(os.environ.get("IAF_CHUNK_W", "1176"))  # columns/chunk (divides 4704)
N_CHUNKS = F // CHUNK_W
X_BUFS = int(os.environ.get("IAF_X_BUFS", "12"))
S_BUFS = int(os.environ.get("IAF_S_BUFS", "8"))
SPIKE_ENGINE = os.environ.get("IAF_SPIKE", "scalar")  # "vector" | "scalar"
OUT_DMA_ENGINE = os.environ.get("IAF_OUT_DMA", "gpsimd")   # scalar|sync|gpsimd
MEM0_DMA_ENGINE = os.environ.get("IAF_MEM0_DMA", "scalar")  # scalar|sync|gpsimd
# On-device output dtype. Spikes are exactly {0,1}, which bf16/uint8 represent
# exactly, so the DRAM output can be narrow and the host upcasts to f32 —
# halving (bf16) or quartering (uint8) HBM write traffic with zero error.
U8 = mybir.dt.uint8
OUT_DT = {"f32": F32, "bf16": BF16, "u8": U8}[os.environ.get("IAF_OUT_DT", "u8")]
# Spike tile dtype in SBUF. "out" = same as OUT_DT (plain copy out-DMA);
# "f32" = keep f32 tiles (fast DVE sub operand) and let the SWDGE out-DMA
# cast down to OUT_DT.
SPIKE_TILE = os.environ.get("IAF_SPIKE_TILE", "out")
# Engine balance: every k-th spike (over the flat (t, c) index) computes as
# is_gt on DVE; the rest as sign+relu on ACT. 0 = all on ACT.
DVE_SPIKE_EVERY = int(os.environ.get("IAF_DVE_SPIKE_EVERY", "16"))
# Split the t=0 loads/ops into half-chunks to shorten the pipeline ramp.
T0_SPLIT = os.environ.get("IAF_T0_SPLIT", "1") == "1"
# Negated-state formulation: membrane tiles carry m-tilde = -mem between
# steps; u = x - m-tilde (TT), m-tilde' = spike - u via scalar_tensor_tensor
# reading the f32 sign scratch — DVE never touches the u8 spike tiles.
# Exact: spike - u and x - m-tilde round identically to the reference ops.
NEG_STATE = os.environ.get("IAF_NEG", "0") == "1"

_compiled_nc = None


def _build():
    nc = bacc.Bacc("TRN2", target_bir_lowering=False, debug=False,
                   num_devices=NCORES)
    # mem0 is folded into x[0] host-side (bit-exact: same fp32 add the device
    # would do), so the device reads only x and writes only spikes.
    x = nc.dram_tensor("x", [T, P, F], F32, kind="ExternalInput").ap()
    out = nc.dram_tensor("out", [T, P, F], OUT_DT, kind="ExternalOutput").ap()

    with tile.TileContext(nc) as tc:
        with tc.tile_pool(name="mem", bufs=N_CHUNKS) as mem_pool, \
             tc.tile_pool(name="xin", bufs=X_BUFS) as x_pool, \
             tc.tile_pool(name="sgn", bufs=4) as sg_pool, \
             tc.tile_pool(name="spk", bufs=S_BUFS) as s_pool:
            eng = {"scalar": nc.scalar, "sync": nc.sync, "gpsimd": nc.gpsimd}
            mem0_dma = eng[MEM0_DMA_ENGINE]
            if SPIKE_ENGINE == "scalar":
                # bias tile for Sign(u - 1); built on DVE inside the tile
                # graph so no manual barrier is needed.
                neg1 = mem_pool.tile([P, 1], F32)
                nc.vector.memset(neg1[:], -1.0)
            # t=0 membrane state is x'[0] = x[0] + mem0 (folded on host):
            # DMA it straight into the resident membrane tiles. With T0_SPLIT
            # the loads go in half-chunks alternating between the two HWDGE
            # queues so the first spike ops can start a few us earlier.
            mts = []
            for c in range(N_CHUNKS):
                mt = mem_pool.tile([P, CHUNK_W], F32)
                if T0_SPLIT:
                    hw = CHUNK_W // 2
                    for h in range(2):
                        dma = mem0_dma if (2 * c + h) % 2 == 0 else nc.sync
                        dma.dma_start(
                            out=mt[:, bass.ts(h, hw)],
                            in_=x[0, :, c * CHUNK_W + h * hw:
                                  c * CHUNK_W + (h + 1) * hw])
                else:
                    mem0_dma.dma_start(out=mt[:],
                                       in_=x[0, :, bass.ts(c, CHUNK_W)])
                mts.append(mt)
            out_dma = eng.get(OUT_DMA_ENGINE)
            # t-outer, chunk-inner, phase-grouped issue order: engines run
            # their instruction streams in order, so grouping each phase
            # across chunks keeps every engine stall-free (chunk c's spike
            # computes while chunk c+1 adds, etc).
            for t in range(T):
                sts = []
                if t > 0:
                    xts = []
                    for c in range(N_CHUNKS):
                        xt = x_pool.tile([P, CHUNK_W], F32)
                        nc.sync.dma_start(out=xt[:],
                                          in_=x[t, :, bass.ts(c, CHUNK_W)])
                        xts.append(xt)
                    for c in range(N_CHUNKS):
                        if NEG_STATE:
                            # u = x - m-tilde  (== x + mem)
                            nc.vector.tensor_sub(mts[c][:], xts[c][:], mts[c][:])
                        else:
                            nc.vector.tensor_add(mts[c][:], mts[c][:], xts[c][:])
                st_dt = F32 if SPIKE_TILE == "f32" else OUT_DT
                # Work items: (membrane AP, output col base, width). At t=0
                # with T0_SPLIT they are half-chunks so the pipeline ramps up
                # as soon as the first half-load lands.
                if t == 0 and T0_SPLIT:
                    hw = CHUNK_W // 2
                    items = [(mts[c][:, bass.ts(h, hw)], c * CHUNK_W + h * hw,
                              hw) for c in range(N_CHUNKS) for h in range(2)]
                else:
                    items = [(mts[c][:], c * CHUNK_W, CHUNK_W)
                             for c in range(N_CHUNKS)]
                sgs = []
                for i, (mem_ap, col, w) in enumerate(items):
                    st = s_pool.tile([P, w], st_dt)
                    sts.append(st)
                    # Offload every k-th spike to DVE, aligned to the END of
                    # each per-t spike phase (last chunk) so the swap never
                    # stalls ACT's stream mid-phase.
                    idx = t * N_CHUNKS + i
                    on_act = SPIKE_ENGINE == "scalar" and not (
                        DVE_SPIKE_EVERY
                        and (idx + 1) % DVE_SPIKE_EVERY == 0)
                    if on_act:
                        # spike = relu(sign(mem' - 1)) in {0,1}, exactly
                        # (u > 1); both steps on the otherwise-idle ScalarE.
                        # Sign goes to an f32 scratch (its {-1,0,1} range
                        # would wrap in uint8); Relu writes the spike dtype.
                        sg = sg_pool.tile([P, w], F32)
                        nc.scalar.activation(
                            sg[:], mem_ap, mybir.ActivationFunctionType.Sign,
                            bias=neg1[:], scale=1.0)
                        nc.scalar.activation(
                            st[:], sg[:], mybir.ActivationFunctionType.Relu)
                        sgs.append(sg)
                    else:
                        nc.vector.tensor_scalar(
                            out=st[:], in0=mem_ap, scalar1=1.0, scalar2=None,
                            op0=mybir.AluOpType.is_gt)
                        sgs.append(None)
                for i, (mem_ap, col, w) in enumerate(items):
                    if NEG_STATE:
                        if sgs[i] is not None:
                            # m-tilde = max(sg, 0) - u = spike - u
                            nc.vector.scalar_tensor_tensor(
                                out=mem_ap, in0=sgs[i][:], scalar=0.0,
                                in1=mem_ap, op0=mybir.AluOpType.max,
                                op1=mybir.AluOpType.subtract)
                        else:
                            # m-tilde = (u > 1) - u
                            nc.vector.scalar_tensor_tensor(
                                out=mem_ap, in0=mem_ap, scalar=1.0,
                                in1=mem_ap, op0=mybir.AluOpType.is_gt,
                                op1=mybir.AluOpType.subtract)
                    else:
                        nc.vector.tensor_sub(mem_ap, mem_ap, sts[i][:])
                    if OUT_DMA_ENGINE == "split":
                        dma_eng = nc.sync if i % 2 == 0 else nc.scalar
                    elif OUT_DMA_ENGINE == "mix":
                        dma_eng = nc.gpsimd if i % 2 == 0 else nc.sync
                    else:
                        dma_eng = out_dma
                    dma_eng.dma_start(out=out[t, :, col:col + w],
                                      in_=sts[i][:])
    nc.compile()
    return nc


def _get_nc():
    global _compiled_nc
    if _compiled_nc is None:
        _compiled_nc = _build()
    return _compiled_nc


def _run(x, mem0, trace=False):
    nc = _get_nc()
    in_maps = []
    for i in range(NCORES):
        bsl = slice(i * BPC, (i + 1) * BPC)
        xi = np.ascontiguousarray(x[:, bsl]).reshape(T, P, F)
        # Fold the initial membrane into the first timestep (bit-exact fp32
        # add, same rounding the device add would produce).
        xi[0] += mem0[bsl].reshape(P, F)
        in_maps.append({"x": xi})
    res = run_bass_kernel_spmd(nc, in_maps, list(range(NCORES)), trace=trace)
    full = np.empty((T, B, C, H, W), dtype=np.float32)
    for i in range(NCORES):
        shard = res.results[i]["out"].reshape(T, BPC, C, H, W)
        full[:, i * BPC:(i + 1) * BPC] = shard  # upcasts narrow spike dtypes
    return full, res


def kernel(x, mem0):
    x = np.asarray(x, dtype=np.float32)
    mem0 = np.asarray(mem0, dtype=np.float32)
    full, _ = _run(x, mem0, trace=False)
    return full
